# revision 27
# baseline (speedup 1.0000x reference)
"""Causal self-attention with RoPE on 8 Trainium2 NeuronCores.

Problem (hardcoded): B=4, S=2048, D=1024, H=16 heads, hd=64, fp32.
  qkv = x @ w_qkv.T ; rope(q, k) ; causal softmax(q k^T / sqrt(hd)) @ v ; out @ w_out.T

Sharding: core c -> (batch b = c//2, head-group hg = c%2 of 8 heads).
Each core computes a full [S, D] partial output (its heads' contribution to
the output projection); the host sums the two partials per batch.

v2 layout highlights (vs. the phase-separated v1):
  - Single fused loop: for each 512-token s-chunk, project Q/K (+RoPE,
    pipelined per chunk via SBUF swap-DMAs) and V, then immediately run
    attention for q-chunk qi == sc. Act(exp) overlaps PE(projection of the
    next chunk); no serialized RoPE phase.
  - Scores as S^T = K^T-tile.T @ Q^T-chunk (per head pair, two 64-row PE
    groups); exp on Act with diagonal tiles trimmed to visible columns.
  - Causal mask applied only to the 128x128 diagonal block of each diagonal
    tile via a DVE multiply with a host-provided 0/1 triangle.
  - A@V computed transposed: out[q, hd] accumulated per 128-q block with
    N=65 (64 v cols + appended ones column giving softmax denominators),
    normalized per-partition with tensor_scalar, then PE-transposed back to
    [hd, q] for the output projection.
  - All matmul operands bf16; f32 PSUM accumulation throughout.
"""

import sys

if "/opt/trn_rl_repo" not in sys.path:
    sys.path.insert(0, "/opt/trn_rl_repo")

import numpy as np

B, S, D = 4, 2048, 1024
H, HD = 16, 64
NCORES = 8
ROPE_BASE = 10000.0

SC = 512          # s-chunk (projection granularity) == q-chunk (attention)
NSC = S // SC     # 4 chunks
NP = 4            # head-pair blocks per core (8 heads)
ND = D // 128     # contraction d-tiles
NST = S // 128    # k/v 128-row s-tiles


class Cfg:
    def __init__(self):
        pass


def build_nc(cfg=None):
    """Build the per-core Bass program (SPMD: same program on all 8 cores)."""
    from contextlib import ExitStack

    import concourse.bass as bass
    from concourse import bacc, mybir, tile

    f32 = mybir.dt.float32
    bf16 = mybir.dt.bfloat16
    Exp = mybir.ActivationFunctionType.Exp

    scale = float(HD) ** -0.5

    nc = bacc.Bacc("TRN2", target_bir_lowering=False, debug=False)

    xT = nc.dram_tensor("xt", [D, S], bf16, kind="ExternalInput").ap()
    wqk = nc.dram_tensor("wqk", [D, 2 * NP * 128], bf16, kind="ExternalInput").ap()
    wv = nc.dram_tensor("wv", [D, NP * 128], bf16, kind="ExternalInput").ap()
    wo = nc.dram_tensor("wo", [NP * 128, D], bf16, kind="ExternalInput").ap()
    cosT = nc.dram_tensor("cost", [128, S], bf16, kind="ExternalInput").ap()
    sinT = nc.dram_tensor("sint", [128, S], bf16, kind="ExternalInput").ap()
    tri = nc.dram_tensor("tri", [128, 128], bf16, kind="ExternalInput").ap()
    iden = nc.dram_tensor("iden", [128, 128], bf16, kind="ExternalInput").ap()
    perm = nc.dram_tensor("perm", [128, 128], bf16, kind="ExternalInput").ap()
    out = nc.dram_tensor("out", [S, D], bf16, kind="ExternalOutput").ap()

    with tile.TileContext(nc) as tc, ExitStack() as ctx:
        persist = ctx.enter_context(tc.tile_pool(name="persist", bufs=1))

        # ---- persistent SBUF ----
        wqk_sb = [persist.tile([128, 2 * NP * 128], bf16, tag=f"wqk{d}", name=f"wqk{d}") for d in range(ND)]
        qt = [persist.tile([128, S], bf16, tag=f"qt{p}", name=f"qt{p}") for p in range(NP)]
        kt = [persist.tile([128, S], bf16, tag=f"kt{p}", name=f"kt{p}") for p in range(NP)]
        vt = [persist.tile([128, 2 * NP * (HD + 1)], bf16, tag=f"vt{si}", name=f"vt{si}") for si in range(NST)]
        wv_sb = [persist.tile([128, NP * 128], bf16, tag=f"wv{d}", name=f"wv{d}") for d in range(ND)]
        wo_sb = [persist.tile([128, D], bf16, tag=f"wo{c}", name=f"wo{c}") for c in range(NP)]
        cos_sb = persist.tile([128, S], bf16, tag="cos")
        sin_sb = persist.tile([128, S], bf16, tag="sin")
        tri_sb = persist.tile([128, 128], bf16, tag="tri")
        iden_sb = persist.tile([128, 128], bf16, tag="iden")
        perm_sb = persist.tile([128, 128], bf16, tag="perm")

        with (
            tc.tile_pool(name="xts", bufs=2) as xp,
            tc.tile_pool(name="traw", bufs=3) as trp,
            tc.tile_pool(name="tmp", bufs=3) as tmpp,
            tc.tile_pool(name="at", bufs=NST + 14) as aw,
            tc.tile_pool(name="nrm", bufs=3) as nrmp,
            tc.tile_pool(name="rec", bufs=4) as recp,
            tc.tile_pool(name="ot", bufs=3) as otp_sb,
            tc.tile_pool(name="ob", bufs=3) as obp,
            tc.tile_pool(name="stp", bufs=3, space="PSUM") as stp,
            tc.tile_pool(name="avp", bufs=1, space="PSUM") as avp,
            tc.tile_pool(name="tpp", bufs=1, space="PSUM") as tpp,
        ):
            # first matmul chain needs wqk + x(sc=0): interleave those DMAs
            # so the d=0 accumulation can start as soon as possible
            first_x = []
            for d_i in range(ND):
                nc.sync.dma_start(wqk_sb[d_i][:], wqk[d_i * 128 : (d_i + 1) * 128, :])
                t = xp.tile([128, SC], bf16, tag=f"x{d_i}", name=f"x{d_i}")
                nc.sync.dma_start(t[:], xT[d_i * 128 : (d_i + 1) * 128, 0:SC])
                first_x.append(t)
            nc.sync.dma_start(cos_sb[:], cosT)
            nc.sync.dma_start(sin_sb[:], sinT)
            nc.sync.dma_start(tri_sb[:], tri)
            nc.sync.dma_start(iden_sb[:], iden)
            nc.sync.dma_start(perm_sb[:], perm)
            for d in range(ND):
                nc.sync.dma_start(wv_sb[d][:], wv[d * 128 : (d + 1) * 128, :])
            for c in range(NP):
                nc.sync.dma_start(wo_sb[c][:], wo[c * 128 : (c + 1) * 128, :])
            for si in range(NST):
                ones_cols = vt[si].rearrange("p (h c) -> p h c", c=HD + 1)[:, :, HD : HD + 1]
                nc.gpsimd.memset(ones_cols, 1.0)

            # Deferred output projection: chains for chunk qi-1 are emitted
            # interleaved into chunk qi's scores loop so they fill the PE
            # stalls while Act chews on exp tiles.
            pending_po = []

            def queue_outproj(qi, ot):
                for dj in range(2):
                    for qs in range(SC // 128):
                        pending_po.append((qi, dj, qs, ot))

            def emit_po(qi, dj, qs, ot):
                q0 = qi * SC + qs * 128
                po = stp.tile([128, 2 * SC], f32, tag="st", name="po")[:, 0:512]
                for c in range(NP):
                    nc.tensor.matmul(
                        po[:],
                        ot[c][:, qs * 128 : (qs + 1) * 128],
                        wo_sb[c][:, dj * 512 : (dj + 1) * 512],
                        start=(c == 0),
                        stop=(c == NP - 1),
                    )
                ob = obp.tile([128, 512], bf16, tag="ob")
                nc.vector.tensor_copy(ob[:], po[:])
                nc.sync.dma_start(out[q0 : q0 + 128, dj * 512 : (dj + 1) * 512], ob[:])

            def emit_one_po():
                if pending_po:
                    emit_po(*pending_po.pop(0))

            def make_proj_units(sc_i):
                """Closures emitting chunk sc_i's projection piecewise: x-DMA,
                then Q/K/V chains (RoPE swap-matmul deferred one unit so it
                never waits on the DVE copy in PE program order)."""
                xts = []
                ccos = cos_sb[:, sc_i * SC : (sc_i + 1) * SC]
                csin = sin_sb[:, sc_i * SC : (sc_i + 1) * SC]
                rope_tail = []

                def dma_x():
                    if sc_i == 0:
                        xts.extend(first_x)
                        return
                    for d_i in range(ND):
                        t = xp.tile([128, SC], bf16, tag=f"x{d_i}", name=f"x{d_i}")
                        nc.sync.dma_start(
                            t[:], xT[d_i * 128 : (d_i + 1) * 128, sc_i * SC : (sc_i + 1) * SC]
                        )
                        xts.append(t)

                def emit_qk(g):
                    ps = stp.tile([128, 2 * SC], f32, tag="st", name="psqk")
                    for d_i in range(ND):
                        nc.tensor.matmul(
                            ps[:, 0:SC],
                            wqk_sb[d_i][:, g * 128 : (g + 1) * 128],
                            xts[d_i][:],
                            start=(d_i == 0),
                            stop=(d_i == ND - 1),
                        )
                    dst = (qt[g] if g < NP else kt[g - NP])[:, sc_i * SC : (sc_i + 1) * SC]
                    # RoPE: dst = raw*cos + swap(raw)*sin (signs baked into
                    # sinT). The partition swap runs on PE as a permutation
                    # matmul into the second half of the same PSUM tile.
                    traw = trp.tile([128, SC], bf16, tag="traw")
                    nc.vector.tensor_copy(traw[:], ps[:, 0:SC])
                    nc.vector.tensor_mul(dst, traw[:], ccos)

                    def tail():
                        nc.tensor.matmul(ps[:, SC : 2 * SC], perm_sb[:], traw[:], start=True, stop=True)
                        tmp = tmpp.tile([128, SC], bf16, tag="tmp")
                        nc.vector.tensor_mul(tmp[:], ps[:, SC : 2 * SC], csin)
                        nc.vector.tensor_add(dst, dst, tmp[:])
                    rope_tail.append(tail)

                def emit_v(ss):
                    ps = stp.tile([128, 2 * SC], f32, tag="st", name="psv")
                    for d_i in range(ND):
                        nc.tensor.matmul(
                            ps[:, 0:SC],
                            xts[d_i][:, ss * 128 : (ss + 1) * 128],
                            wv_sb[d_i][:],
                            start=(d_i == 0),
                            stop=(d_i == ND - 1),
                        )
                    si = sc_i * (SC // 128) + ss
                    v_cols = vt[si].rearrange("p (h c) -> p h c", c=HD + 1)[:, :, 0:HD]
                    nc.vector.tensor_copy(v_cols, ps[:, 0:SC].rearrange("p (h c) -> p h c", c=HD))

                def unit(kind, idx):
                    def go():
                        if kind == "v":
                            emit_v(idx)
                        else:
                            emit_qk(idx if kind == "q" else NP + idx)
                        if len(rope_tail) > 1:
                            rope_tail.pop(0)()
                    return go

                def flush_tails():
                    while rope_tail:
                        rope_tail.pop(0)()

                units = [dma_x]
                for kind, idx in [
                    ("q", 0), ("k", 0), ("v", 0), ("q", 1), ("k", 1), ("v", 1),
                    ("q", 2), ("k", 2), ("v", 2), ("q", 3), ("k", 3), ("v", 3),
                ]:
                    units.append(unit(kind, idx))
                units.append(flush_tails)
                return units

            # chunk 0's projection runs standalone (DMA-paced anyway); later
            # chunks' projections interleave into the previous chunk's
            # attention score loop to fill the PE idle behind Act's exp rate
            for u in make_proj_units(0):
                u()

            # ---------- attention: generator per q-chunk ----------
            # Score tiles are produced by a generator; a driver interleaves
            # projection units, deferred outproj chains, and the work queue
            # (A@V+normalize, then transpose) between tiles. Once the next
            # chunk's projection is fully emitted, the driver pre-pulls that
            # chunk's score tiles so the Act engine's exp pipeline never
            # drains, even in the last chunk's tail.
            work = []
            ots = {}

            def make_av(qi, pb, qs, ats, ot):
                def go():
                    g = qi * (SC // 128) + qs
                    avt = avp.tile([128, 2 * (HD + 1)], f32, tag="avt")
                    for ph in range(2):
                        h = 2 * pb + ph
                        for ki in range(g + 1):
                            nc.tensor.matmul(
                                avt[:, ph * (HD + 1) : (ph + 1) * (HD + 1)],
                                ats[ki][:, ph * SC + qs * 128 : ph * SC + (qs + 1) * 128],
                                vt[ki][:, h * (HD + 1) : (h + 1) * (HD + 1)],
                                start=(ki == 0),
                                stop=(ki == g),
                            )
                    nrm = nrmp.tile([128, 128], bf16, tag="nrm")
                    for ph in range(2):
                        rec = recp.tile([128, 1], f32, tag="rec")
                        nc.vector.reciprocal(
                            rec[:], avt[:, ph * (HD + 1) + HD : ph * (HD + 1) + HD + 1]
                        )
                        nc.vector.tensor_scalar_mul(
                            nrm[:, ph * HD : (ph + 1) * HD],
                            avt[:, ph * (HD + 1) : ph * (HD + 1) + HD],
                            rec[:],
                        )
                    work.append((state["gt"], make_tp(qi, pb, qs, nrm, ot)))
                return go

            def make_tp(qi, pb, qs, nrm, ot):
                def go():
                    tp = tpp.tile([128, 128], bf16, tag="tp")
                    nc.tensor.transpose(tp[:], nrm[:], iden_sb[:])
                    nc.vector.tensor_copy(ot[pb][:, qs * 128 : (qs + 1) * 128], tp[:])
                    if qi == NSC - 1 and pb == NP - 1:
                        # last chunk: output projection inline per q-block
                        for dj in range(2):
                            emit_po(qi, dj, qs, ot)
                return go

            def attention_gen(qi):
                nkt = (qi + 1) * (SC // 128)
                ot = [
                    otp_sb.tile([128, SC], bf16, tag=f"ot{p}", name=f"ot{p}")
                    for p in range(NP)
                ]
                ots[qi] = ot
                for pb in range(NP):
                    ats = []
                    for ki in range(nkt):
                        j = ki - qi * (SC // 128)  # >=0: diagonal tile index
                        trim = 128 * j if j >= 0 else 0
                        st = stp.tile([128, 2 * SC], f32, tag="st", name="st")
                        for ph in range(2):
                            prow = ph * 64
                            nc.tensor.matmul(
                                st[:, ph * SC + trim : (ph + 1) * SC],
                                kt[pb][prow : prow + 64, ki * 128 : (ki + 1) * 128],
                                qt[pb][prow : prow + 64, qi * SC + trim : (qi + 1) * SC],
                                start=True,
                                stop=True,
                            )
                        at = aw.tile([128, 2 * SC], bf16, tag="at")
                        if j < 0:
                            nc.scalar.activation(at[:], st[:], Exp, scale=scale)
                        else:
                            for ph in range(2):
                                sl = slice(ph * SC + trim, (ph + 1) * SC)
                                nc.scalar.activation(at[:, sl], st[:, sl], Exp, scale=scale)
                            # zero the strict upper triangle of the 128x128
                            # diagonal block (cols qs == j)
                            for ph in range(2):
                                blk = slice(ph * SC + trim, ph * SC + trim + 128)
                                nc.gpsimd.tensor_mul(at[:, blk], at[:, blk], tri_sb[:])
                        ats.append(at)
                        # A@V for q-block qs becomes ready two tiles after
                        # its last score tile (ki == 4*qi + qs), leaving slack
                        # for the exp and diagonal-mask chain
                        if j >= 2:
                            work.append(make_av(qi, pb, j - 2, ats, ot))
                        yield
                    work.append(make_av(qi, pb, (SC // 128) - 2, ats, ot))
                    work.append(make_av(qi, pb, (SC // 128) - 1, ats, ot))

            def pull(g):
                try:
                    next(g)
                    return True
                except StopIteration:
                    return False

            pulled = {qi: 0 for qi in range(NSC)}
            cur = attention_gen(0)
            for qi in range(NSC):
                units = make_proj_units(qi + 1) if qi + 1 < NSC else []
                nxt = None
                total = (qi + 1) * (SC // 128) * NP - pulled[qi]
                stride = max(1, (2 * total // 3) // (len(units) + 1)) if units else 0
                po_stride = max(1, total // 9)
                t = 0
                while pull(cur):
                    t += 1
                    if units and stride and t % stride == 0:
                        units.pop(0)()
                    if pending_po and t % po_stride == 0:
                        emit_one_po()
                    if work:
                        work.pop(0)()
                    if len(work) > 4:
                        work.pop(0)()
                    if not units and qi < NSC - 1 and pulled[qi + 1] < 12 and t % 2 == 0:
                        if nxt is None:
                            nxt = attention_gen(qi + 1)
                        if pull(nxt):
                            pulled[qi + 1] += 1
                # boundary: drain this chunk's remaining work, filling with
                # next-chunk score tiles where possible
                while units:
                    units.pop(0)()
                while work:
                    if qi < NSC - 1:
                        if nxt is None:
                            nxt = attention_gen(qi + 1)
                        if pull(nxt):
                            pulled[qi + 1] += 1
                    work.pop(0)()
                while pending_po:
                    emit_one_po()
                if qi < NSC - 1:
                    queue_outproj(qi, ots[qi])
                    cur = nxt if nxt is not None else attention_gen(qi + 1)

    nc.compile()
    return nc


def rope_tables(Sz: int):
    """cosT [128, S] and sign-baked sinT [128, S] in the [hd, s] layout.

    q' = q * cosT + swap(q) * sinT, where swap exchanges partition halves
    (0:32 <-> 32:64) within each 64-row block.
    """
    inv_freq = 1.0 / (ROPE_BASE ** (np.arange(0, HD, 2, dtype=np.float32) / HD))
    t = np.arange(Sz, dtype=np.float32)
    freqs = t[:, None] * inv_freq[None, :]  # [S, 32]
    emb = np.concatenate([freqs, freqs], axis=-1)  # [S, 64]
    cos = np.cos(emb).T.astype(np.float32)  # [64, S]
    sin = np.sin(emb).T.astype(np.float32)  # [64, S]
    sin2 = sin.copy()
    sin2[0:32] = -sin2[0:32]
    cosT = np.concatenate([cos, cos], axis=0)
    sinT = np.concatenate([sin2, sin2], axis=0)
    return np.ascontiguousarray(cosT), np.ascontiguousarray(sinT)


def core_inputs(x, w_qkv, w_out, core: int):
    """Host-side prep of one core's input map."""
    import ml_dtypes

    ndt = ml_dtypes.bfloat16
    b, hg = core // 2, core % 2
    Dz = x.shape[2]
    hpc_rows = (H // 2) * HD  # 512 rows per head-group
    r0 = hg * hpc_rows
    wq = w_qkv[r0 : r0 + hpc_rows, :]
    wk = w_qkv[Dz + r0 : Dz + r0 + hpc_rows, :]
    wv_ = w_qkv[2 * Dz + r0 : 2 * Dz + r0 + hpc_rows, :]
    cosT, sinT = rope_tables(x.shape[1])
    tri = (np.arange(128)[None, :] >= np.arange(128)[:, None]).astype(ndt)  # keep c >= p
    iden = np.eye(128, dtype=ndt)
    # partition swap for rotate_half: 32-blocks 0<->1 and 2<->3
    sw = np.arange(128)
    sw = (sw // 32 ^ 1) * 32 + sw % 32
    pm = np.zeros((128, 128), dtype=ndt)
    pm[sw, np.arange(128)] = 1.0  # out[m] = in[sw[m]]
    return {
        "xt": np.ascontiguousarray(x[b].T).astype(ndt),
        "wqk": np.ascontiguousarray(np.concatenate([wq, wk], axis=0).T).astype(ndt),
        "wv": np.ascontiguousarray(wv_.T).astype(ndt),
        "wo": np.ascontiguousarray(w_out[:, r0 : r0 + hpc_rows].T).astype(ndt),
        "cost": cosT.astype(ndt),
        "sint": sinT.astype(ndt),
        "tri": tri,
        "iden": iden,
        "perm": pm,
    }


_CACHE = {}


def kernel(x, w_qkv, w_out):
    x = np.asarray(x, dtype=np.float32)
    w_qkv = np.asarray(w_qkv, dtype=np.float32)
    w_out = np.asarray(w_out, dtype=np.float32)
    assert x.shape == (B, S, D) and w_qkv.shape == (3 * D, D) and w_out.shape == (D, D)

    from concourse.bass_utils import run_bass_kernel_spmd

    if "nc" not in _CACHE:
        _CACHE["nc"] = build_nc(Cfg())
    nc = _CACHE["nc"]

    in_maps = [core_inputs(x, w_qkv, w_out, c) for c in range(NCORES)]
    res = run_bass_kernel_spmd(nc, in_maps, core_ids=list(range(NCORES)))
    outs = [np.asarray(res.results[c]["out"], dtype=np.float32) for c in range(NCORES)]
    full = np.empty((B, S, D), dtype=np.float32)
    for b in range(B):
        full[b] = outs[2 * b] + outs[2 * b + 1]
    return full
